# revision 4
# baseline (speedup 1.0000x reference)
"""Trainium2 Bass kernel for DiscreteTimeS4.

Reference computation (per batch element b):
    x_proj = relu(x @ Wi^T + bi)          [T, P]
    u      = x_proj @ B                   [T, H]
    h_t    = a * h_{t-1} + u_t            (diagonal linear scan over T)
    y      = hs @ C                       [T, P]
    out    = y @ Wo^T + bo                [T, O]

Sharding: data-parallel over the batch axis — core b handles x_seq[b].
Weights replicated. No cross-device communication.

Device strategy (per core, all matmuls in fp32r = full-rate PE):
  - host pre-transposes x to x^T [D, T] so the contraction dim (D) sits on
    SBUF partitions; host also fuses W2 = C @ Wo^T so stages 4+5 collapse
    into one matmul.
  - pipeline over time chunks of 512:
      MM1: XP^T[p,t] = Wi @ x^T          (lhsT = Wi^T, K=512)
      ACT: relu(psum + bi) -> SBUF (fp32r)
      MM2: U^T[h,t] = B^T @ XP^T         (lhsT = B, K=256)
      DVE: tensor_tensor_scan: h = a*h + u along t (fp32 state, carry
           chained across chunks via initial=prev[:, -1:])
      MM3: OUT[t,o] = (HS^T)^T @ W2 + bo (lhsT = HS^T tile -> natural [t,o]
           output layout, no transpose on store; bo folded in as a K=1
           matmul of ones^T @ bo_row)
      copy psum -> SBUF (ACT / DVE split), DMA out.
"""

import numpy as np

try:
    import concourse.bass as bass  # noqa: F401
except ImportError:  # pragma: no cover
    import sys

    sys.path.insert(0, "/opt/trn_rl_repo")

from contextlib import ExitStack

import concourse.mybir as mybir
import concourse.tile as tile
from concourse import bacc
from concourse.bass import ts
from concourse.bass_utils import run_bass_kernel_spmd

BSZ, T, D, P, H, O = 8, 4096, 512, 256, 256, 512
CH = 512  # time-chunk (free dim of MM1/MM2, PSUM bank = 512 fp32)
NCH = T // CH
F32 = mybir.dt.float32
F32R = mybir.dt.float32r

# per chunk: how many of the 4 output t-subtiles take the DVE bias-add path
# (remaining go through a K=1 bias matmul on PE + plain ACT copy)
N_DVE_BIAS = 2

_NC_CACHE = {}


def build_nc(mm_dt=F32R, n_dve_bias=N_DVE_BIAS, nch=NCH):
    key = (mm_dt, n_dve_bias, nch)
    if key in _NC_CACHE:
        return _NC_CACHE[key]
    wdt = mm_dt
    nc = bacc.Bacc("TRN2", target_bir_lowering=False, debug=False)

    xT_d = nc.dram_tensor("xT", [D, T], wdt, kind="ExternalInput")
    wiT_d = nc.dram_tensor("wiT", [D, P], wdt, kind="ExternalInput")
    bmat_d = nc.dram_tensor("bmat", [P, H], wdt, kind="ExternalInput")
    w2_d = nc.dram_tensor("w2", [H, O], wdt, kind="ExternalInput")
    arep_d = nc.dram_tensor("arep", [H, CH], F32, kind="ExternalInput")
    bicol_d = nc.dram_tensor("bicol", [P, 1], F32, kind="ExternalInput")
    borep_d = nc.dram_tensor("borep", [128, O], F32, kind="ExternalInput")
    ones_d = nc.dram_tensor("ones", [1, 128], wdt, kind="ExternalInput")
    borow_d = nc.dram_tensor("borow", [1, O], wdt, kind="ExternalInput")
    out_d = nc.dram_tensor("out", [T, O], F32, kind="ExternalOutput")

    KD = D // 128  # 4 k-tiles for MM1
    KP = P // 128  # 2
    KH = H // 128  # 2
    MO = O // 128  # 4 out t-subtiles per chunk

    with tile.TileContext(nc) as tc, ExitStack() as ctx:
        wpool = ctx.enter_context(tc.tile_pool(name="weights", bufs=1))
        xpool = ctx.enter_context(tc.tile_pool(name="x", bufs=3))
        xppool = ctx.enter_context(tc.tile_pool(name="xp", bufs=2))
        hspool = ctx.enter_context(tc.tile_pool(name="hs", bufs=3))
        opool = ctx.enter_context(tc.tile_pool(name="osb", bufs=3))
        psA = ctx.enter_context(tc.tile_pool(name="psA", bufs=2, space="PSUM"))
        psB = ctx.enter_context(tc.tile_pool(name="psB", bufs=2, space="PSUM"))
        psO = ctx.enter_context(tc.tile_pool(name="psO", bufs=4, space="PSUM"))

        # ---- load weights/constants once
        wiT_sb = wpool.tile([128, KD, P], wdt)
        nc.sync.dma_start(
            out=wiT_sb, in_=wiT_d.ap().rearrange("(k p) m -> p k m", p=128)
        )
        bmat_sb = wpool.tile([128, KP, H], wdt)
        nc.sync.dma_start(
            out=bmat_sb, in_=bmat_d.ap().rearrange("(k p) m -> p k m", p=128)
        )
        w2_sb = wpool.tile([128, KH, O], wdt)
        nc.sync.dma_start(
            out=w2_sb, in_=w2_d.ap().rearrange("(k p) m -> p k m", p=128)
        )
        arep_sb = wpool.tile([128, KH, CH], F32)
        nc.sync.dma_start(
            out=arep_sb, in_=arep_d.ap().rearrange("(k p) m -> p k m", p=128)
        )
        bicol_sb = wpool.tile([128, KP, 1], F32)
        nc.sync.dma_start(
            out=bicol_sb, in_=bicol_d.ap().rearrange("(k p) m -> p k m", p=128)
        )
        borep_sb = wpool.tile([128, O], F32)
        nc.sync.dma_start(out=borep_sb, in_=borep_d.ap())
        ones_sb = wpool.tile([1, 128], wdt)
        nc.sync.dma_start(out=ones_sb, in_=ones_d.ap())
        borow_sb = wpool.tile([1, O], wdt)
        nc.sync.dma_start(out=borow_sb, in_=borow_d.ap())

        xT_v = xT_d.ap().rearrange("(k p) t -> p k t", p=128)
        out_v = out_d.ap().rearrange("(c s p) o -> c p s o", p=128, s=MO)

        prev_hs = None
        for c in range(nch):
            tsl = slice(c * CH, (c + 1) * CH)

            # ---- load x^T chunk [128, KD, CH]
            x_sb = xpool.tile([128, KD, CH], wdt)
            nc.sync.dma_start(out=x_sb, in_=xT_v[:, :, tsl])

            # ---- MM1 + relu/bias -> xp_sb
            xp_sb = xppool.tile([128, KP, CH], wdt)
            for m in range(KP):
                ps1 = psA.tile([128, CH], F32, tag="ps1")
                for k in range(KD):
                    nc.tensor.matmul(
                        ps1[:, :],
                        wiT_sb[:, k, ts(m, 128)],
                        x_sb[:, k, :],
                        start=(k == 0),
                        stop=(k == KD - 1),
                    )
                nc.scalar.activation(
                    out=xp_sb[:, m, :],
                    in_=ps1[:, :],
                    func=mybir.ActivationFunctionType.Relu,
                    bias=bicol_sb[:, m, :],
                    scale=1.0,
                )

            # ---- MM2 + scan -> hs_sb
            hs_sb = hspool.tile([128, KH, CH], wdt)
            for m in range(KH):
                ps2 = psB.tile([128, CH], F32, tag="ps2")
                for k in range(KP):
                    nc.tensor.matmul(
                        ps2[:, :],
                        bmat_sb[:, k, ts(m, 128)],
                        xp_sb[:, k, :],
                        start=(k == 0),
                        stop=(k == KP - 1),
                    )
                init = 0.0 if prev_hs is None else prev_hs[:, m, CH - 1 : CH]
                nc.vector.tensor_tensor_scan(
                    out=hs_sb[:, m, :],
                    data0=arep_sb[:, m, :],
                    data1=ps2[:, :],
                    initial=init,
                    op0=mybir.AluOpType.mult,
                    op1=mybir.AluOpType.add,
                )
            prev_hs = hs_sb

            # ---- MM3 (+bo) -> out
            o_sb = opool.tile([128, MO, O], F32)
            for st in range(MO):
                ps3 = psO.tile([128, O], F32, tag="ps3")
                use_pe_bias = st >= n_dve_bias
                for k in range(KH):
                    nc.tensor.matmul(
                        ps3[:, :],
                        hs_sb[:, k, ts(st, 128)],
                        w2_sb[:, k, :],
                        start=(k == 0),
                        stop=(k == KH - 1 and not use_pe_bias),
                    )
                if use_pe_bias:
                    nc.tensor.matmul(
                        ps3[:, :],
                        ones_sb[:, :],
                        borow_sb[:, :],
                        start=False,
                        stop=True,
                    )
                    nc.scalar.copy(o_sb[:, st, :], ps3[:, :])
                else:
                    nc.vector.tensor_add(o_sb[:, st, :], ps3[:, :], borep_sb[:, :])

            nc.sync.dma_start(out=out_v[c], in_=o_sb)

    nc.finalize()
    _NC_CACHE[key] = nc
    return nc


def _prep_shared(a, B, C, Wi, bi, Wo, bo):
    w2 = (C.astype(np.float64) @ Wo.astype(np.float64).T).astype(np.float32)
    shared = {
        "wiT": np.ascontiguousarray(Wi.T),
        "bmat": np.ascontiguousarray(B),
        "w2": np.ascontiguousarray(w2),
        "arep": np.ascontiguousarray(np.broadcast_to(a[:, None], (H, CH))),
        "bicol": np.ascontiguousarray(bi[:, None]),
        "borep": np.ascontiguousarray(np.broadcast_to(bo[None, :], (128, O))),
        "ones": np.ones((1, 128), dtype=np.float32),
        "borow": np.ascontiguousarray(bo[None, :]),
    }
    return shared


def kernel(x_seq, a, B, C, Wi, bi, Wo, bo, _collect=None):
    nc = build_nc()
    shared = _prep_shared(a, B, C, Wi, bi, Wo, bo)
    in_maps = []
    for b in range(BSZ):
        m = dict(shared)
        m["xT"] = np.ascontiguousarray(x_seq[b].T)
        in_maps.append(m)
    res = run_bass_kernel_spmd(
        nc,
        in_maps,
        core_ids=list(range(BSZ)),
        **(_collect or {}),
    )
    if _collect is not None:
        _collect["res"] = res
    out = np.stack([res.results[b]["out"] for b in range(BSZ)], axis=0)
    return out
